# revision 12
# baseline (speedup 1.0000x reference)
"""KNN top-16 kernel for Trainium2 (8 NeuronCores, SPMD).

Problem (hardcoded): p1 (4,8192,3) f32, p2 (4,8192,3) f32, lengths1/2 (4,) i32.
Returns (idx int64 (4,8192,16), dists f32 (4,8192,16)) matching
jax.lax.top_k(-sq_dists, 16) semantics with PyTorch3D-style padding.

Sharding: core c handles batch n=c//2, query rows [(c%2)*4096, (c%2+1)*4096).
p2 of that batch is replicated to the core (per the sharding hint).

Device algorithm per 128-query tile (v2 — chunked two-scan selection):
  One fp16 matmul per 512-target chunk computes
    s[i,j] = 2*p1_i.p2_j - ||p2_j||^2 - 60000*(j >= len2)
  via a hi/lo fp16 split (12 contraction rows: ah.bh + ah.bl + al.bh
  - q2h - q2l - mask), accurate to ~3e-5 abs — top-16 selection then
  differs from fp32 only on near-ties (~28 of 512K indices).
  DVE reads each PSUM 1024-wide selection cell (2 banks) directly:
    MAX8 -> per-cell top-8 values  (candV [128,64])
    FIND_INDEX8 -> per-cell relative indices (candI [128,64] u16)
  Merge on 64 candidates: MAX8/FIND_INDEX8/MATCH_REPLACE8/MAX8/
  FIND_INDEX8 -> top-16 values + candidate positions. Exactness: the
  global top-16 is covered unless >8 of it falls in one 1024-cell;
  with fp16-split tie noise included, 69 of 512K indices differ from
  the fp32 reference (idx rel-err 9.2e-3, within the 2e-2 gate).
  candI ships to host; host computes idx = (pos>>3)*1024 + candI[pos]
  and dists = ||p1||^2 - v.

All per-core inputs are packed into one DRAM tensor (one DMA, one
semaphore) because TRN2 instructions support at most 2 sync waits and
Tile does not collapse transitive cross-queue waits.
"""

import numpy as np
from functools import lru_cache

N, P1, P2, D, K = 4, 8192, 8192, 3, 16
N_CORES = 8
QPC = P1 // 2          # queries per core (4096)
TILE = 128             # query rows per tile
NTILES = QPC // TILE   # 32
CHUNK = 512            # matmul free-dim chunk == PSUM bank
CELL = 1024            # selection cell (two PSUM banks per MAX8 scan)
NCELL = P2 // CELL     # 8
NCAND = NCELL * 8      # 64 candidates per query
ROWS = 12              # fp16 contraction rows
MASKV = np.float32(60000.0)
INW = QPC + P2         # packed input width per partition (12288)


@lru_cache(maxsize=1)
def _build_program():
    from concourse.bass import Bass
    from concourse.tile import TileContext
    import concourse.mybir as mybir

    f32 = mybir.dt.float32
    f16 = mybir.dt.float16
    u16 = mybir.dt.uint16

    nc = Bass("TRN2", num_devices=N_CORES)

    inp_d = nc.dram_tensor("inp", [ROWS, INW], f16, kind="ExternalInput")
    # p-major staging layout: [p, t*K+k]; host permutes to [t*128+p, k].
    val_d = nc.dram_tensor("val_out", [TILE, NTILES * K], f32, kind="ExternalOutput")
    pos_d = nc.dram_tensor("pos_out", [TILE, NTILES * K], u16, kind="ExternalOutput")
    ci_d = nc.dram_tensor("ci_out", [TILE, NTILES * NCAND], u16, kind="ExternalOutput")

    with TileContext(nc) as tc:
        with tc.tile_pool(name="const", bufs=1) as cpool, \
             tc.tile_pool(name="cand", bufs=2) as spool, \
             tc.tile_pool(name="psum", bufs=4, space="PSUM") as ppool:
            inp_sb = cpool.tile([ROWS, INW], f16)
            # Input DMAs split and ordered by first-need time (same queue =>
            # serial in issue order): tile-0's lhsT, then cells 0-2, then the
            # remaining queries, then cells 3-7. Compute starts ~10us earlier
            # than with one monolithic transfer.
            def span(a, b):
                nc.sync.dma_start(inp_sb[:, a:b], inp_d[:, a:b])
            span(0, TILE)                                  # stat tile 0
            for c in range(3):                             # cells 0-2
                span(QPC + c * CELL, QPC + (c + 1) * CELL)
            span(TILE, QPC)                                # stat tiles 1-31
            for c in range(3, NCELL):                      # cells 3-7
                span(QPC + c * CELL, QPC + (c + 1) * CELL)
            stat_sb = inp_sb[:, 0:QPC]
            mov_sb = inp_sb[:, QPC:INW]
            # Persistent result staging: each region written exactly once,
            # so DVE writes carry no slot-reuse deps; two DMAs at the end.
            val_st = cpool.tile([TILE, NTILES * K], f32)
            pos_st = cpool.tile([TILE, NTILES * K], u16)

            for t in range(NTILES):
                candV = spool.tile([TILE, NCAND], f32, tag="candV")
                candI = spool.tile([TILE, NCAND], u16, tag="candI")
                # 1-element dummy write absorbs candI's pool slot-reuse wait
                # (its last reader is the Sync-engine DMA) so the real cell
                # ops each carry only the PE-semaphore wait. candV's last
                # reader is DVE itself - program order already covers it.
                nc.vector.memset(candI[:, 0:1], 0)
                lhsT = stat_sb[:, t * TILE:(t + 1) * TILE]
                for c in range(NCELL):
                    ps = ppool.tile([TILE, CELL], f32, tag="ps")
                    for half in range(CELL // CHUNK):
                        j0 = c * CELL + half * CHUNK
                        nc.tensor.matmul(
                            ps[:, half * CHUNK:(half + 1) * CHUNK], lhsT,
                            mov_sb[:, j0:j0 + CHUNK],
                            start=True, stop=True,
                        )
                    cv = candV[:, c * 8:(c + 1) * 8]
                    nc.vector.max(out=cv, in_=ps)
                    nc.vector.max_index(
                        out=candI[:, c * 8:(c + 1) * 8], in_max=cv,
                        in_values=ps)

                v0 = val_st[:, t * K:t * K + 8]
                v1 = val_st[:, t * K + 8:(t + 1) * K]
                nc.vector.max(out=v0, in_=candV)
                nc.vector.max_index(
                    out=pos_st[:, t * K:t * K + 8], in_max=v0, in_values=candV)
                nc.vector.match_replace(
                    out=candV, in_to_replace=v0, in_values=candV,
                    imm_value=-1e38,
                )
                nc.vector.max(out=v1, in_=candV)
                nc.vector.max_index(
                    out=pos_st[:, t * K + 8:(t + 1) * K], in_max=v1,
                    in_values=candV)
                nc.sync.dma_start(ci_d[:, t * NCAND:(t + 1) * NCAND], candI)
                sk = slice(t * K, (t + 1) * K)
                nc.sync.dma_start(val_d[:, sk], val_st[:, sk])
                nc.sync.dma_start(pos_d[:, sk], pos_st[:, sk])

    # This walrus build allows only ~1 sync wait per instruction; the
    # framework tail Drain carries one wait per busy proc. Split all but
    # the last wait onto single-wait NoOps chained before it (same engine,
    # program order => identical blocking semantics).
    import concourse.mybir as mb
    fix = 0
    for fn in nc.m.functions:
        for blk in fn.blocks:
            insts = blk.instructions
            i = 0
            while i < len(insts):
                inst = insts[i]
                si = inst.sync_info
                if si is not None and len(si.on_wait) > 1:
                    head, last = si.on_wait[:-1], si.on_wait[-1:]
                    pre = []
                    for w in head:
                        fix += 1
                        nop = mb.InstNoOp(name=f"I-waitfix-{fix}", ins=[],
                                          outs=[])
                        nop.engine = inst.engine
                        nop.sync_info = mb.SyncInfo(on_wait=[w], on_update=[])
                        pre.append(nop)
                    si.on_wait = last
                    insts[i:i] = pre
                    i += len(pre)
                i += 1
    return nc


def _f16(x):
    return np.asarray(x, np.float16)


def _core_inputs(p1, p2, lengths2, core):
    n, h = core // 2, core % 2
    q0 = h * QPC
    a = 2.0 * p1[n, q0:q0 + QPC]       # (4096, 3) f32
    b = p2[n]                           # (8192, 3) f32

    ah = _f16(a)
    al = _f16(a - ah.astype(np.float32))
    bh = _f16(b)
    bl = _f16(b - bh.astype(np.float32))
    q2 = (b.astype(np.float64) ** 2).sum(1).astype(np.float32)
    q2h = _f16(q2)
    q2l = _f16(q2 - q2h.astype(np.float32))
    mask = np.where(np.arange(P2) >= lengths2[n], MASKV, np.float32(0.0))

    inp = np.empty((ROWS, INW), np.float16)
    stat = inp[:, 0:QPC]
    mov = inp[:, QPC:INW]
    stat[0:3] = ah.T
    stat[3:6] = ah.T
    stat[6:9] = al.T
    stat[9:12] = np.float16(-1.0)
    mov[0:3] = bh.T
    mov[3:6] = bl.T
    mov[6:9] = bh.T
    mov[9] = q2h
    mov[10] = q2l
    mov[11] = _f16(mask)
    return {"inp": inp}


def kernel(p1, p2, lengths1, lengths2):
    from concourse.bass_utils import run_bass_kernel_spmd

    p1 = np.asarray(p1, np.float32)
    p2 = np.asarray(p2, np.float32)
    lengths1 = np.asarray(lengths1, np.int32)
    lengths2 = np.asarray(lengths2, np.int32)

    nc = _build_program()
    in_maps = [_core_inputs(p1, p2, lengths2, c) for c in range(N_CORES)]
    res = run_bass_kernel_spmd(nc, in_maps, core_ids=list(range(N_CORES)))

    # host epilogue: dists = ||p1||^2 - s, idx recovery, pad-row zeroing
    p1sq = (p1[:, :, 0] * p1[:, :, 0] + p1[:, :, 1] * p1[:, :, 1]) \
        + p1[:, :, 2] * p1[:, :, 2]                      # (4, 8192) f32

    dists = np.zeros((N, P1, K), np.float32)
    idx = np.zeros((N, P1, K), np.int64)
    for c in range(N_CORES):
        n, h = c // 2, c % 2
        sl = slice(h * QPC, (h + 1) * QPC)
        v = res.results[c]["val_out"].reshape(TILE, NTILES, K)
        pos = res.results[c]["pos_out"].reshape(TILE, NTILES, K)
        ci = res.results[c]["ci_out"].reshape(TILE, NTILES, NCAND)
        v = v.transpose(1, 0, 2).reshape(QPC, K)
        pos = pos.transpose(1, 0, 2).reshape(QPC, K).astype(np.int64)
        ci = ci.transpose(1, 0, 2).reshape(QPC, NCAND)
        rel = np.take_along_axis(ci, pos, axis=1).astype(np.int64)
        dists[n, sl] = p1sq[n, sl, None] - v
        idx[n, sl] = (pos >> 3) * CELL + rel

    for n in range(N):
        L = int(lengths1[n])
        dists[n, L:] = 0.0
        idx[n, L:] = 0
    return idx, dists


# revision 17
# speedup vs baseline: 1.0474x; 1.0474x over previous
"""KNN top-16 kernel for Trainium2 (8 NeuronCores, SPMD).

Problem (hardcoded): p1 (4,8192,3) f32, p2 (4,8192,3) f32, lengths1/2 (4,) i32.
Returns (idx int64 (4,8192,16), dists f32 (4,8192,16)) matching
jax.lax.top_k(-sq_dists, 16) semantics with PyTorch3D-style padding.

Sharding: core c handles batch n=c//2, query rows [(c%2)*4096, (c%2+1)*4096).
p2 of that batch is replicated to the core (per the sharding hint).

Device algorithm per 128-query tile (v2 — chunked two-scan selection):
  One fp16 matmul per 512-target chunk computes
    s[i,j] = 2*p1_i.p2_j - ||p2_j||^2 - 60000*(j >= len2)
  via a hi/lo fp16 split (12 contraction rows: ah.bh + ah.bl + al.bh
  - q2h - q2l - mask), accurate to ~3e-5 abs — top-16 selection then
  differs from fp32 only on near-ties (~28 of 512K indices).
  DVE reads each PSUM 1024-wide selection cell (2 banks) directly:
    MAX8 -> per-cell top-8 values  (candV [128,64])
    FIND_INDEX8 -> per-cell relative indices (candI [128,64] u16)
  Both candidate arrays ship to host per tile (the DMA engines have
  ~10x slack); the host merges 64 candidates/query to top-16 with a
  stable argsort, reproducing the device tie order. The DVE thus runs
  ONLY the two irreducible scans per cell. Exactness: the global
  top-16 is covered unless >8 of it falls in one 1024-cell; with
  fp16-split tie noise included, 69 of 512K indices differ from the
  fp32 reference (idx rel-err 9.2e-3, within the 2e-2 gate). Host:
  idx = (slot>>3)*1024 + candI[slot], dists = ||p1||^2 - v.

All per-core inputs are packed into one DRAM tensor (one DMA, one
semaphore) because TRN2 instructions support at most 2 sync waits and
Tile does not collapse transitive cross-queue waits.
"""

import numpy as np
from functools import lru_cache

N, P1, P2, D, K = 4, 8192, 8192, 3, 16
N_CORES = 8
QPC = P1 // 2          # queries per core (4096)
TILE = 128             # query rows per tile
NTILES = QPC // TILE   # 32
CHUNK = 512            # matmul free-dim chunk == PSUM bank
CELL = 1024            # selection cell (two PSUM banks per MAX8 scan)
NCELL = P2 // CELL     # 8
NCAND = NCELL * 8      # 64 candidates per query
ROWS = 12              # fp16 contraction rows
MASKV = np.float32(60000.0)
INW = QPC + P2         # packed input width per partition (12288)


@lru_cache(maxsize=1)
def _build_program():
    from concourse.bass import Bass
    from concourse.tile import TileContext
    import concourse.mybir as mybir

    f32 = mybir.dt.float32
    f16 = mybir.dt.float16
    u16 = mybir.dt.uint16

    nc = Bass("TRN2", num_devices=N_CORES)

    inp_d = nc.dram_tensor("inp", [ROWS, INW], f16, kind="ExternalInput")
    # p-major staging layout: [p, t*NCAND+s]; host permutes to [t*128+p, s].
    cv_d = nc.dram_tensor("cv_out", [TILE, NTILES * NCAND], f32, kind="ExternalOutput")
    ci_d = nc.dram_tensor("ci_out", [TILE, NTILES * NCAND], u16, kind="ExternalOutput")

    with TileContext(nc) as tc:
        with tc.tile_pool(name="const", bufs=1) as cpool, \
             tc.tile_pool(name="cand", bufs=2) as spool, \
             tc.tile_pool(name="psum", bufs=4, space="PSUM") as ppool:
            inp_sb = cpool.tile([ROWS, INW], f16)
            # Input DMAs split and ordered by first-need time (same queue =>
            # serial in issue order): tile-0's lhsT, then cells 0-2, then the
            # remaining queries, then cells 3-7. Compute starts ~10us earlier
            # than with one monolithic transfer.
            def span(a, b):
                nc.sync.dma_start(inp_sb[:, a:b], inp_d[:, a:b])
            span(0, TILE)                                  # stat tile 0
            for c in range(3):                             # cells 0-2
                span(QPC + c * CELL, QPC + (c + 1) * CELL)
            span(TILE, QPC)                                # stat tiles 1-31
            for c in range(3, NCELL):                      # cells 3-7
                span(QPC + c * CELL, QPC + (c + 1) * CELL)
            stat_sb = inp_sb[:, 0:QPC]
            mov_sb = inp_sb[:, QPC:INW]

            for t in range(NTILES):
                candV = spool.tile([TILE, NCAND], f32, tag="candV")
                candI = spool.tile([TILE, NCAND], u16, tag="candI")
                # 1-element dummy writes absorb the pool slot-reuse waits
                # (both tiles' last readers are Sync-engine DMAs) so the
                # real cell ops each carry only the PE-semaphore wait.
                nc.vector.memset(candV[:, 0:1], 0.0)
                nc.vector.memset(candI[:, 0:1], 0)
                lhsT = stat_sb[:, t * TILE:(t + 1) * TILE]
                for c in range(NCELL):
                    ps = ppool.tile([TILE, CELL], f32, tag="ps")
                    for half in range(CELL // CHUNK):
                        j0 = c * CELL + half * CHUNK
                        nc.tensor.matmul(
                            ps[:, half * CHUNK:(half + 1) * CHUNK], lhsT,
                            mov_sb[:, j0:j0 + CHUNK],
                            start=True, stop=True,
                        )
                    cv = candV[:, c * 8:(c + 1) * 8]
                    nc.vector.max(out=cv, in_=ps)
                    nc.vector.max_index(
                        out=candI[:, c * 8:(c + 1) * 8], in_max=cv,
                        in_values=ps)

                sc = slice(t * NCAND, (t + 1) * NCAND)
                nc.sync.dma_start(cv_d[:, sc], candV)
                nc.sync.dma_start(ci_d[:, sc], candI)

    # This walrus build allows only ~1 sync wait per instruction; the
    # framework tail Drain carries one wait per busy proc. Split all but
    # the last wait onto single-wait NoOps chained before it (same engine,
    # program order => identical blocking semantics).
    import concourse.mybir as mb
    fix = 0
    for fn in nc.m.functions:
        for blk in fn.blocks:
            insts = blk.instructions
            i = 0
            while i < len(insts):
                inst = insts[i]
                si = inst.sync_info
                if si is not None and len(si.on_wait) > 1:
                    head, last = si.on_wait[:-1], si.on_wait[-1:]
                    pre = []
                    for w in head:
                        fix += 1
                        nop = mb.InstNoOp(name=f"I-waitfix-{fix}", ins=[],
                                          outs=[])
                        nop.engine = inst.engine
                        nop.sync_info = mb.SyncInfo(on_wait=[w], on_update=[])
                        pre.append(nop)
                    si.on_wait = last
                    insts[i:i] = pre
                    i += len(pre)
                i += 1
    return nc


def _f16(x):
    return np.asarray(x, np.float16)


def _core_inputs(p1, p2, lengths2, core):
    n, h = core // 2, core % 2
    q0 = h * QPC
    a = 2.0 * p1[n, q0:q0 + QPC]       # (4096, 3) f32
    b = p2[n]                           # (8192, 3) f32

    ah = _f16(a)
    al = _f16(a - ah.astype(np.float32))
    bh = _f16(b)
    bl = _f16(b - bh.astype(np.float32))
    q2 = (b.astype(np.float64) ** 2).sum(1).astype(np.float32)
    q2h = _f16(q2)
    q2l = _f16(q2 - q2h.astype(np.float32))
    mask = np.where(np.arange(P2) >= lengths2[n], MASKV, np.float32(0.0))

    inp = np.empty((ROWS, INW), np.float16)
    stat = inp[:, 0:QPC]
    mov = inp[:, QPC:INW]
    stat[0:3] = ah.T
    stat[3:6] = ah.T
    stat[6:9] = al.T
    stat[9:12] = np.float16(-1.0)
    mov[0:3] = bh.T
    mov[3:6] = bl.T
    mov[6:9] = bh.T
    mov[9] = q2h
    mov[10] = q2l
    mov[11] = _f16(mask)
    return {"inp": inp}


def kernel(p1, p2, lengths1, lengths2):
    from concourse.bass_utils import run_bass_kernel_spmd

    p1 = np.asarray(p1, np.float32)
    p2 = np.asarray(p2, np.float32)
    lengths1 = np.asarray(lengths1, np.int32)
    lengths2 = np.asarray(lengths2, np.int32)

    nc = _build_program()
    in_maps = [_core_inputs(p1, p2, lengths2, c) for c in range(N_CORES)]
    res = run_bass_kernel_spmd(nc, in_maps, core_ids=list(range(N_CORES)))

    # host epilogue: dists = ||p1||^2 - s, idx recovery, pad-row zeroing
    p1sq = (p1[:, :, 0] * p1[:, :, 0] + p1[:, :, 1] * p1[:, :, 1]) \
        + p1[:, :, 2] * p1[:, :, 2]                      # (4, 8192) f32

    dists = np.zeros((N, P1, K), np.float32)
    idx = np.zeros((N, P1, K), np.int64)
    for c in range(N_CORES):
        n, h = c // 2, c % 2
        sl = slice(h * QPC, (h + 1) * QPC)
        cv = res.results[c]["cv_out"].reshape(TILE, NTILES, NCAND)
        ci = res.results[c]["ci_out"].reshape(TILE, NTILES, NCAND)
        cv = cv.transpose(1, 0, 2).reshape(QPC, NCAND)
        ci = ci.transpose(1, 0, 2).reshape(QPC, NCAND)
        # top-16 of 64 candidates; stable sort = device tie order
        # (value desc, then lower slot = lower cell = lower index)
        slot = np.argsort(-cv, axis=1, kind="stable")[:, :K]
        selV = np.take_along_axis(cv, slot, axis=1)
        rel = np.take_along_axis(ci, slot, axis=1).astype(np.int64)
        dists[n, sl] = p1sq[n, sl, None] - selV
        idx[n, sl] = (slot >> 3) * CELL + rel

    for n in range(N):
        L = int(lengths1[n])
        dists[n, L:] = 0.0
        idx[n, L:] = 0
    return idx, dists
